# revision 43
# baseline (speedup 1.0000x reference)
"""Trainium2 Bass kernel for the AESUELOGIT segment-reduce problem.

Strategy (8 NeuronCores, SPMD):
  - Shard the 20000 paths across cores ALIGNED TO OD BOUNDARIES, with the
    cuts chosen to EQUALIZE per-core path counts.  The segmented softmax is
    then fully core-local: no denominator collective is needed.
  - Path chunks (128 paths) are ALSO od-aligned (host packs whole ods into
    chunks, padding the remainder), so the same-od denominator matmul is a
    single per-chunk B matmul -- no cross-chunk stitch matmuls.
  - Head: features arrive host-transposed (link-partition, bf16); the
    link-utility lhsT is built by DVE ops k-group-pipelined with the feature
    DMA.
  - matmul1: bf16 utilities against an fp8 D (0/1 incidence is exact in
    e4m3).  Per 128-path chunk: transpose -> exp (bias adds the theta_links
    row, written straight to bf16) -> denominator via the in-chunk B matmul
    -> reciprocal_approx_fast -> flows (stt straight to fp8) -> matmul2 as
    fp8 DoubleRow over chunk PAIRS into 4 PSUM banks.
  - Cross-core reduction of the (96, 2048) partial link flows is an
    ALL-TO-ALL (pure transport; ~2.3x faster than ReduceScatter on this
    fabric) followed by a 7-op on-core add tree in the folded (96, 256)
    layout, split across the vector and gpsimd queues.
  - Each core runs the BPR epilogue on its 12 day-hour rows in the folded
    (96, 256) layout; the host concatenates.
"""

import os

import numpy as np
import ml_dtypes

import concourse.bacc as bacc
import concourse.bass as bass
import concourse.mybir as mybir
import concourse.tile as tile
from concourse.bass_utils import run_bass_kernel_spmd

F32 = mybir.dt.float32
BF16 = mybir.dt.bfloat16
F8 = mybir.dt.float8e4
AF = mybir.ActivationFunctionType
ALU = mybir.AluOpType
DR = mybir.MatmulPerfMode.DoubleRow

ND, NH, NL, NF = 4, 24, 2000, 4
NP, NOD, NCORES = 20000, 4000, 8
DH = ND * NH            # 96
DHS = DH // NCORES      # 12 day-hour rows per core after the exchange
L_PAD = 2048            # links padded to 16*128
KL = L_PAD // 128       # 16 link chunks
KG = 4                  # k-groups for the head pipeline (4 chunks each)
KPG = KL // KG
FB = L_PAD // 256       # 8 link blocks in the folded epilogue layout
EPS = 1e-12


def _build_program(PSHARD):
    """Emit the SPMD Bass program (identical on all cores)."""
    NCH = PSHARD // 128                  # even (host guarantees)
    NPS = (PSHARD + 511) // 512          # 512-path blocks
    NPR = NCH // 2                       # chunk pairs for fp8 DoubleRow mm2
    nc = bacc.Bacc("TRN2", target_bir_lowering=False, debug=False,
                   num_devices=NCORES)

    # ---- parameters (per-core shards) ----
    # features transposed: [link%128, (kg, f, kk, dh)] bf16
    p_ft = nc.dram_tensor("ftt", [128, KG * NF * KPG * DH], BF16,
                          kind="ExternalInput")
    # D block-major fp8: p_d[b][p, 512k + j] = D[128k+p, 512b+j]
    p_d = nc.dram_tensor("dkb", [NPS, 128, KL * 512], F8,
                         kind="ExternalInput")
    # D^T chunk-pair-major fp8: p_dt[t][p, c*L_PAD + l], c in {0,1}
    p_dt = nc.dram_tensor("dtp", [NPR, 128, 2 * L_PAD], F8,
                          kind="ExternalInput")
    # same-od 0/1 matrices, one per od-aligned chunk
    p_B = nc.dram_tensor("bod", [128, NCH * 128], F8, kind="ExternalInput")
    p_qs = nc.dram_tensor("qsp", [128, NCH], F32, kind="ExternalInput")
    # theta_links chunks (cols 0..KL-1) + theta_raw replicated (cols KL..)
    p_tc = nc.dram_tensor("tc", [128, KL + NF], F32, kind="ExternalInput")
    p_id = nc.dram_tensor("idn", [128, 128], F32, kind="ExternalInput")
    # folded (96, 256) epilogue tensors (host-replicated layouts, no math)
    p_kb = nc.dram_tensor("kb96", [DH, 256], F32, kind="ExternalInput")
    p_bb = nc.dram_tensor("bb96", [DH, 256], F32, kind="ExternalInput")
    p_lab = nc.dram_tensor("lab96", [DH, 256], F32, kind="ExternalInput")
    p_ttf = nc.dram_tensor("ttf96", [DH, 256], F32, kind="ExternalInput")
    p_out = nc.dram_tensor("out", [DH, 256], F32, kind="ExternalOutput")
    groups = [list(range(NCORES))]

    with tile.TileContext(nc) as tc:
        with tc.tile_pool(name="const", bufs=1) as cpool, \
             tc.tile_pool(name="dram", bufs=1, space="DRAM") as dpool, \
             tc.tile_pool(name="big", bufs=1) as bpool, \
             tc.tile_pool(name="stream", bufs=4) as spool, \
             tc.tile_pool(name="dkp", bufs=3) as dkpool, \
             tc.tile_pool(name="dtp", bufs=4) as dtpool:

            # warmup AllGather, fully dependency-free so it executes in
            # the quiet window BEFORE the DMA ring ramps (CC ops yield to
            # DMA traffic on this fabric -- with any late-landing input
            # the warmup floats to compute end and serializes in front of
            # the real AllToAll; without it the AllToAll pays ~20us of
            # first-collective cost)
            wgs = cpool.tile([1, 32], F32, tag="wgs")
            nc.vector.memset(wgs[:], 0.0)
            wg_in = dpool.tile([1, 32], F32, tag="wgi")
            nc.sync.dma_start(wg_in[:], wgs[:])
            wg_out = dpool.tile([1, 32 * NCORES], F32, tag="wgo")
            nc.gpsimd.collective_compute(
                "AllGather", ALU.bypass, replica_groups=groups,
                ins=[wg_in.opt()], outs=[wg_out.opt()])

            # ---- DMA ring.  mm1 block 0 is gated on dk0 + the g0 lhsT,
            # so the scalar queue carries exactly that critical prefix.
            # The sync ring is GATED on dk0's arrival (tiny dummy read)
            # and the gpsimd ring on dk1's, so the early wire belongs to
            # the critical transfers alone.
            tc_sb = cpool.tile([128, KL + NF], F32, tag="tc")
            nc.scalar.dma_start(tc_sb[:], p_tc.ap())
            GW = NF * KPG * DH               # cols per k-group in p_ft
            fts = bpool.tile([128, KG * GW], BF16, tag="fts")
            nc.scalar.dma_start(fts[:, 0:GW], p_ft.ap()[:, 0:GW])

            dk_t = [dkpool.tile([128, KL * 512], F8, tag="dk",
                                name=f"dk{b}") for b in range(NPS)]
            dtp_t = [dtpool.tile([128, 2 * L_PAD], F8, tag="dt",
                                 name=f"dt{t}") for t in range(NPR)]
            nc.scalar.dma_start(dk_t[0][:], p_d.ap()[0])
            ident = cpool.tile([128, 128], F32, tag="ident")
            nc.scalar.dma_start(ident[:], p_id.ap())
            nc.scalar.dma_start(dtp_t[0][:], p_dt.ap()[0])
            nc.scalar.dma_start(fts[:, GW:2 * GW], p_ft.ap()[:, GW:2 * GW])
            nc.scalar.dma_start(dtp_t[1][:], p_dt.ap()[1])

            # sync ring: gate on the g0 feature slab (small, lands early)
            # so dk0 owns the first wire window, then deadline order
            gate0 = dpool.tile([1, 32], BF16, tag="gate0")
            nc.sync.dma_start(gate0[:], fts[0:1, 0:32])
            if NPS > 1:
                nc.sync.dma_start(dk_t[1][:], p_d.ap()[1])
            b_all = cpool.tile([128, NCH * 128], F8, tag="b_all")
            nc.sync.dma_start(b_all[:, 0:NCH * 64], p_B.ap()[:, 0:NCH * 64])
            qs_sb = cpool.tile([128, NCH], F32, tag="qs")
            nc.sync.dma_start(qs_sb[:], p_qs.ap())
            for g in range(2, KG):
                nc.sync.dma_start(fts[:, g * GW:(g + 1) * GW],
                                  p_ft.ap()[:, g * GW:(g + 1) * GW])
            nc.sync.dma_start(b_all[:, NCH * 64:], p_B.ap()[:, NCH * 64:])
            kb = cpool.tile([DH, 256], F32, tag="kb")
            nc.sync.dma_start(kb[:], p_kb.ap())
            bb = cpool.tile([DH, 256], F32, tag="bb")
            nc.sync.dma_start(bb[:], p_bb.ap())
            lab = cpool.tile([DH, 256], F32, tag="lab")
            nc.sync.dma_start(lab[:], p_lab.ap())
            ttf = cpool.tile([DH, 256], F32, tag="ttf")
            nc.sync.dma_start(ttf[:], p_ttf.ap())
            # remaining dk blocks and dt pairs, ordered by use deadline,
            # issued from the (otherwise idle) gpsimd queue; gated on the
            # g1 feature slab so the early wire stays uncontended
            gate1 = dpool.tile([1, 32], BF16, tag="gate1")
            nc.gpsimd.dma_start(gate1[:], fts[0:1, GW:GW + 32])
            later = [("dk", b) for b in range(2, NPS)]
            for t in range(2, NPR):
                later.insert(min(2 * (t - 2) + 1, len(later)), ("dt", t))
            for kind, i in later:
                if kind == "dk":
                    nc.gpsimd.dma_start(dk_t[i][:], p_d.ap()[i])
                else:
                    nc.gpsimd.dma_start(dtp_t[i][:], p_dt.ap()[i])

            thb = cpool.tile([128, NF], F32, tag="thb")
            nc.vector.tensor_scalar_min(thb[:], tc_sb[:, KL:KL + NF], 0.0)

            with tc.tile_pool(name="psA", bufs=2, space="PSUM") as psA:
                # lhsT build, k-group pipelined with the feature DMA:
                # v0t[l, (k, dh)] = sum_f theta_f * featsT (bf16, 2x DVE
                # rate), then packed to lh[l, (k, dh..+tl)] with
                # theta_links in col 96 of each chunk.  Packs for the
                # early groups ride the vector queue (scalar is still
                # issuing DMAs), later groups on scalar.
                v0t = bpool.tile([128, KL * DH], BF16, tag="v0t")
                lh = bpool.tile([128, KL * 97], BF16, tag="lh")
                GV = KPG * DH                # v0t cols per k-group
                for g in range(KG):
                    base = g * GW
                    nc.vector.tensor_scalar_mul(
                        v0t[:, g * GV:(g + 1) * GV],
                        fts[:, base:base + GV], thb[:, 0:1])
                    for f in range(1, NF):
                        nc.vector.scalar_tensor_tensor(
                            v0t[:, g * GV:(g + 1) * GV],
                            fts[:, base + f * GV:base + (f + 1) * GV],
                            thb[:, f:f + 1],
                            v0t[:, g * GV:(g + 1) * GV], ALU.mult, ALU.add)
                    lh3 = lh[:].rearrange("p (k j) -> p k j", j=97)
                    pk = nc.vector.tensor_copy if g < 2 else nc.scalar.copy
                    pk(lh3[:, g * KPG:(g + 1) * KPG, 0:DH],
                       v0t[:, g * GV:(g + 1) * GV].rearrange(
                           "p (k j) -> p k j", j=DH))
                    pk(lh3[:, g * KPG:(g + 1) * KPG, DH:DH + 1],
                       tc_sb[:, g * KPG:(g + 1) * KPG].rearrange(
                           "p (k j) -> p k j", j=1))

                # ---- the block pipeline ----
                ysb = bpool.tile([97, PSHARD], F32, tag="ysb")
                evb = bpool.tile([128, DH * NCH], BF16, tag="evb")
                ft8 = bpool.tile([128, DH * NCH], F8, tag="ft8")
                qsq = cpool.tile([128, NCH], F32, tag="qsq")
                nc.vector.tensor_mul(qsq[:], qs_sb[:], qs_sb[:])
                ar_in = dpool.tile([DH, L_PAD], BF16, tag="arin")
                ar_aa = dpool.tile([DH, L_PAD], BF16, tag="araa")
                # epilogue constants (inputs land ~20us; emitted mid-ring
                # below so the queues never stall on them)
                ib = cpool.tile([DH, 256], F32, tag="ib")
                bb2 = cpool.tile([DH, 256], F32, tag="bb2")
                ab = cpool.tile([DH, 256], F32, tag="ab")
                atf = cpool.tile([DH, 256], F32, tag="atf")

                def chunk_softmax(c):
                    """transpose -> exp for path chunk c."""
                    yt_ps = psA.tile([128, 97], F32, tag="m",
                                     name=f"yt{c}")
                    nc.tensor.matmul(yt_ps[:], ysb[:, 128 * c:128 * (c + 1)],
                                     ident[:97, :97], is_transpose=True,
                                     start=True, stop=True)
                    cvec = spool.tile([128, 1], F32, tag="cvec")
                    nc.vector.tensor_copy(cvec[:], yt_ps[:, DH:DH + 1])
                    nc.scalar.activation(evb[:, DH * c:DH * (c + 1)],
                                         yt_ps[:, 0:DH], AF.Exp, bias=cvec[:])

                def chunk_flow(c):
                    """in-chunk denominator matmul + path flows for chunk c;
                    fp8 DoubleRow matmul2 on each odd c for the pair
                    (c-1, c)."""
                    g_ps = psA.tile([128, DH], F32, tag="m", name=f"g{c}")
                    nc.tensor.matmul(g_ps[:],
                                     b_all[:, 128 * c:128 * (c + 1)],
                                     evb[:, DH * c:DH * (c + 1)],
                                     start=True, stop=True)
                    rec = spool.tile([128, DH], F32, tag="rec")
                    nc.vector.reciprocal_approx_fast(out=rec[:], in_=g_ps[:])
                    nc.vector.scalar_tensor_tensor(
                        ft8[:, DH * c:DH * (c + 1)],
                        evb[:, DH * c:DH * (c + 1)],
                        qsq[:, c:c + 1], rec[:], ALU.mult, ALU.mult)
                    if c % 2 == 1:
                        t = c // 2
                        lhs3 = ft8[:].rearrange(
                            "p (c m) -> p c m", m=DH)[:, c - 1:c + 1, :]
                        rhs3 = dtp_t[t][:].rearrange(
                            "p (c l) -> p c l", c=2)
                        for n in range(L_PAD // 512):
                            nc.tensor.matmul(
                                x_ps[n][:], lhs3,
                                rhs3[:, :, 512 * n:512 * (n + 1)],
                                perf_mode=DR,
                                start=(c == 1), stop=(c == NCH - 1))

                with tc.tile_pool(name="psV", bufs=2, space="PSUM") as psV, \
                     tc.tile_pool(name="psX", bufs=1, space="PSUM") as psX:
                    x_ps = [psX.tile([DH, 512], F32, tag=f"x{n}",
                                     name=f"x{n}")
                            for n in range(L_PAD // 512)]
                    for b in range(NPS):
                        w = min(512, PSHARD - 512 * b)
                        vf_ps = psV.tile([97, w], F32, tag="vf",
                                         name=f"vf{b}")
                        for k in range(KL):
                            nc.tensor.matmul(
                                vf_ps[:], lh[:, 97 * k:97 * (k + 1)],
                                dk_t[b][:, 512 * k:512 * k + w],
                                start=(k == 0), stop=(k == KL - 1))
                        if b % 2 == 0:
                            nc.scalar.copy(ysb[:, 512 * b:512 * b + w],
                                           vf_ps[:])
                        else:
                            nc.vector.tensor_copy(
                                ysb[:, 512 * b:512 * b + w], vf_ps[:])
                        for c in range(4 * b, 4 * b + w // 128):
                            chunk_softmax(c)
                            chunk_flow(c)
                        if b == min(3, NPS - 1):
                            # epilogue constant prep, hidden under compute
                            # (late enough that the const DMAs have landed)
                            nc.vector.reciprocal_approx_fast(
                                out=ib[:], in_=kb[:])
                            nc.vector.tensor_scalar(bb2[:], bb[:],
                                                    float(EPS), 4.0,
                                                    ALU.max, ALU.min)
                            nc.scalar.activation(ab[:], lab[:], AF.Exp)
                            nc.vector.tensor_mul(atf[:], ab[:], ttf[:])

                    # force the Ln table load NOW (hidden under the
                    # drain + AllToAll) instead of on the tail chain
                    scr = spool.tile([128, 1], F32, tag="scr")
                    nc.scalar.activation(scr[:], qs_sb[:, 0:1], AF.Ln)

                    # drain: PSUM -> bf16 -> DRAM -> one AllToAll
                    xb = bpool.tile([DH, L_PAD], BF16, tag="xb")
                    for n in range(L_PAD // 512):
                        if n % 2 == 0:
                            nc.scalar.copy(xb[:, 512 * n:512 * (n + 1)],
                                           x_ps[n][:])
                        else:
                            nc.vector.tensor_copy(
                                xb[:, 512 * n:512 * (n + 1)], x_ps[n][:])
                        nc.sync.dma_start(
                            ar_in[:, 512 * n:512 * (n + 1)],
                            xb[:, 512 * n:512 * (n + 1)])
                    nc.gpsimd.collective_compute(
                        "AllToAll", ALU.bypass,
                        replica_groups=groups,
                        ins=[ar_in.opt()], outs=[ar_aa.opt()])

                # ---- gather the 8 incoming partials in the folded
                # (96, 256) layout (one fold DMA per peer, issue cost
                # split across the sync + scalar queues), then an add
                # tree split across vector + gpsimd ----
                # NOTE: this must stay one DMA per peer with a 3D source
                # AP and a plain 2D dest -- every fancier multi-peer AP
                # form (split dest partition dim, j-outer 3D dest) lowers
                # to silently wrong descriptors on this stack.
                xga = bpool.tile([DH, NCORES * 256], BF16, tag="xga")
                for j in range(NCORES):
                    eng = (nc.sync, nc.gpsimd, nc.scalar)[j % 3]
                    eng.dma_start(
                        xga[:, 256 * j:256 * (j + 1)],
                        ar_aa[DHS * j:DHS * (j + 1), :].rearrange(
                            "d (a l) -> (d a) l", a=FB))
                xs0 = bpool.tile([DH, 256], F32, tag="xs0")
                nc.vector.tensor_add(xs0[:], xga[:, 0:256], xga[:, 256:512])
                for j in (2, 3):
                    nc.vector.tensor_add(
                        xs0[:], xs0[:], xga[:, 256 * j:256 * (j + 1)])
                xs1 = bpool.tile([DH, 256], F32, tag="xs1")
                nc.gpsimd.tensor_add(xs1[:], xga[:, 1024:1280],
                                     xga[:, 1280:1536])
                for j in (6, 7):
                    nc.gpsimd.tensor_add(
                        xs1[:], xs1[:], xga[:, 256 * j:256 * (j + 1)])

                # ---- BPR epilogue in the folded (96, 256) layout ----
                # (max before add is equivalent here: both sides >= 0)
                t0 = bpool.tile([DH, 256], F32, tag="t0")
                nc.vector.scalar_tensor_tensor(
                    t0[:], xs0[:], 1e-35, xs1[:], ALU.max, ALU.add)
                nc.vector.tensor_mul(t0[:], t0[:], ib[:])
                t1 = bpool.tile([DH, 256], F32, tag="t1")
                nc.scalar.activation(t1[:], t0[:], AF.Ln)
                nc.vector.tensor_mul(t1[:], t1[:], bb2[:])
                t2 = bpool.tile([DH, 256], F32, tag="t2")
                nc.scalar.activation(t2[:], t1[:], AF.Exp)
                nc.vector.tensor_mul(t2[:], t2[:], atf[:])
                o_t = bpool.tile([DH, 256], F32, tag="o")
                nc.vector.tensor_add(o_t[:], t2[:], ttf[:])
                nc.sync.dma_start(p_out.ap(), o_t[:])

    nc.compile()
    return nc


_CACHE = {}
LAST_RESULT = None


def _get_program(PSHARD):
    if PSHARD not in _CACHE:
        _CACHE[PSHARD] = _build_program(PSHARD)
    return _CACHE[PSHARD]


def _pack_ods(odl):
    """Best-fit-decreasing packing of whole ods into 128-path chunks.
    Returns (slots, n_chunks): slots[j] is the padded position of local
    path j (odl is sorted, paths of one od contiguous)."""
    uods, ucnts = np.unique(odl, return_counts=True)
    order = np.argsort(-ucnts, kind="stable")
    cap = []                      # remaining capacity per chunk
    place = np.empty((len(uods), 2), np.int64)
    for oi in order:
        cnt = int(ucnts[oi])
        best, bestrem = -1, 129
        for bi, r in enumerate(cap):
            if cnt <= r < bestrem:
                best, bestrem = bi, r
        if best < 0:
            cap.append(128)
            best = len(cap) - 1
        place[oi] = (best, 128 - cap[best])
        cap[best] -= cnt
    starts = np.concatenate([[0], np.cumsum(ucnts)[:-1]])
    slots = np.empty(len(odl), np.int64)
    for oi in range(len(uods)):
        bi, off = place[oi]
        cnt = ucnts[oi]
        slots[starts[oi]:starts[oi] + cnt] = bi * 128 + off + np.arange(cnt)
    return slots, len(cap)


def _fold96(v_lpad):
    """(L_PAD,) per-link vector -> (96, 256) folded layout (row 8*d + a holds
    link block [256a, 256(a+1)) for every local day-hour d)."""
    return np.ascontiguousarray(
        np.tile(v_lpad.reshape(FB, 256), (DHS, 1)).astype(np.float32))


def kernel(X, theta_raw, theta_links, q_sqrt, log_alpha, beta_raw, k, D,
           od_of_path, n_ods):
    X = np.asarray(X, np.float32)
    D = np.asarray(D, np.float32)
    od = np.asarray(od_of_path, np.int32)
    assert X.shape == (ND, NH, NL, NF + 1) and D.shape == (NL, NP)
    assert int(n_ods) == NOD

    # core bounds: equal PATH counts, snapped to od boundaries (keeps the
    # per-core chunk count at its minimum)
    bounds = np.empty(NCORES + 1, np.int64)
    bounds[0], bounds[-1] = 0, NP
    for i in range(1, NCORES):
        idx = i * NP // NCORES
        bounds[i] = np.searchsorted(od, od[min(idx, NP - 1)])

    # ---- od-aligned chunk packing (index bookkeeping only) ----
    slot_maps = []
    nch_need = 0
    for i in range(NCORES):
        slots, nch = _pack_ods(od[bounds[i]:bounds[i + 1]])
        slot_maps.append(slots)
        nch_need = max(nch_need, nch)
    NCH = int(np.ceil(nch_need / 2) * 2)     # even for mm2 pairs
    PSHARD = NCH * 128
    NPS = (PSHARD + 511) // 512

    nc = _get_program(PSHARD)

    F8H = ml_dtypes.float8_e4m3fn

    # ---- host-side shard construction (index bookkeeping + relayout only) --
    Xf = X.reshape(DH, NL, NF + 1)
    ttf_full = np.zeros((DH, L_PAD), np.float32)
    ttf_full[:, :NL] = Xf[:, :, 0]
    # featsT[link%128, (kg, f, kk, dh)] bf16
    ftt = np.zeros((L_PAD, NF, DH), np.float32)
    for f in range(NF):
        ftt[:NL, f, :] = Xf[:, :, f + 1].T
    ftt = (ftt.reshape(KG, KPG, 128, NF, DH).transpose(2, 0, 3, 1, 4)
           .reshape(128, KG * NF * KPG * DH))
    ftt_h = np.ascontiguousarray(ftt).astype(ml_dtypes.bfloat16)

    def padded_vec(v, fill=0.0):
        o = np.full(L_PAD, fill, np.float32)
        o[:NL] = v
        return o

    tc_h = np.concatenate(
        [padded_vec(np.asarray(theta_links, np.float32)).reshape(KL, 128).T,
         np.tile(np.asarray(theta_raw, np.float32), (128, 1))], axis=1)
    tc_h = np.ascontiguousarray(tc_h)
    kb_h = _fold96(padded_vec(np.asarray(k, np.float32), fill=1.0))
    bb_h = _fold96(padded_vec(np.asarray(beta_raw, np.float32)))
    lab_h = _fold96(padded_vec(np.asarray(log_alpha, np.float32)))
    qsr = np.asarray(q_sqrt, np.float32)
    id_h = np.eye(128, dtype=np.float32)

    in_maps = []
    for i in range(NCORES):
        lo, hi = bounds[i], bounds[i + 1]
        odl = od[lo:hi]
        slots = slot_maps[i]

        PB = NPS * 512
        Dsh = np.zeros((L_PAD, PB), np.float32)
        Dsh[:NL, slots] = D[:, lo:hi]
        # block-major D fp8: dkb[b][p, 512k + j] = D[128k+p, 512b+j]
        dkb = np.ascontiguousarray(
            Dsh.reshape(KL, 128, NPS, 512).transpose(2, 1, 0, 3)
            .reshape(NPS, 128, KL * 512)).astype(F8H)
        # chunk-pair-major D^T fp8: dtp[t][p, c*L_PAD + l] = D^T[(2t+c)*128+p, l]
        dtp = np.ascontiguousarray(
            Dsh.T[:PSHARD].astype(F8H).reshape(NCH // 2, 2, 128, L_PAD)
            .transpose(0, 2, 1, 3).reshape(NCH // 2, 128, 2 * L_PAD))

        # same-od 0/1 matrices (pure index bookkeeping); od-aligned chunks
        # mean no od crosses a chunk boundary
        odp = np.full(NCH * 128, -1, np.int64)
        odp[slots] = odl
        oc = odp.reshape(NCH, 128)
        b_h = np.zeros((128, NCH, 128), F8H)
        for c in range(NCH):
            b_h[:, c, :] = (oc[c][:, None] == oc[c][None, :])

        qs_h = np.zeros(NCH * 128, np.float32)
        qs_h[slots] = qsr[odl]
        qs_h = np.ascontiguousarray(qs_h.reshape(NCH, 128).T)

        in_maps.append(dict(
            ftt=ftt_h, dkb=dkb, dtp=dtp,
            bod=np.ascontiguousarray(b_h.reshape(128, NCH * 128)),
            qsp=qs_h, tc=tc_h, idn=id_h,
            kb96=kb_h, bb96=bb_h, lab96=lab_h,
            ttf96=np.ascontiguousarray(
                ttf_full[DHS * i:DHS * (i + 1)].reshape(DH, 256))))

    trace = os.environ.get("BASS_KERNEL_TRACE", "0") == "1"
    global LAST_RESULT
    for _attempt in range(3):
        res = run_bass_kernel_spmd(nc, in_maps, core_ids=list(range(NCORES)),
                                   trace=trace)
        LAST_RESULT = res
        parts = [r["out"].reshape(DHS, L_PAD) for r in res.results]
        out = np.concatenate(parts, axis=0)[:, :NL]
        if np.isfinite(out).all():
            break
    return np.ascontiguousarray(out).reshape(ND, NH, NL).astype(np.float32)


# revision 44
# speedup vs baseline: 1.0185x; 1.0185x over previous
"""Trainium2 Bass kernel for the AESUELOGIT segment-reduce problem.

Strategy (8 NeuronCores, SPMD):
  - Shard the 20000 paths across cores ALIGNED TO OD BOUNDARIES, with the
    cuts chosen to EQUALIZE per-core path counts.  The segmented softmax is
    then fully core-local: no denominator collective is needed.
  - Path chunks (128 paths) are ALSO od-aligned (host packs whole ods into
    chunks, padding the remainder), so the same-od denominator matmul is a
    single per-chunk B matmul -- no cross-chunk stitch matmuls.
  - Head: features arrive host-transposed (link-partition, bf16); the
    link-utility lhsT is built by DVE ops k-group-pipelined with the feature
    DMA.
  - matmul1: bf16 utilities against an fp8 D (0/1 incidence is exact in
    e4m3).  Per 128-path chunk: transpose -> exp (bias adds the theta_links
    row, written straight to bf16) -> denominator via the in-chunk B matmul
    -> reciprocal_approx_fast -> flows (stt straight to fp8) -> matmul2 as
    fp8 DoubleRow over chunk PAIRS into 4 PSUM banks.
  - Cross-core reduction of the (96, 2048) partial link flows is an
    ALL-TO-ALL (pure transport; ~2.3x faster than ReduceScatter on this
    fabric) followed by a 7-op on-core add tree in the folded (96, 256)
    layout, split across the vector and gpsimd queues.
  - Each core runs the BPR epilogue on its 12 day-hour rows in the folded
    (96, 256) layout; the host concatenates.
"""

import os

import numpy as np
import ml_dtypes

import concourse.bacc as bacc
import concourse.bass as bass
import concourse.mybir as mybir
import concourse.tile as tile
from concourse.bass_utils import run_bass_kernel_spmd

F32 = mybir.dt.float32
BF16 = mybir.dt.bfloat16
F8 = mybir.dt.float8e4
AF = mybir.ActivationFunctionType
ALU = mybir.AluOpType
DR = mybir.MatmulPerfMode.DoubleRow

ND, NH, NL, NF = 4, 24, 2000, 4
NP, NOD, NCORES = 20000, 4000, 8
DH = ND * NH            # 96
DHS = DH // NCORES      # 12 day-hour rows per core after the exchange
L_PAD = 2048            # links padded to 16*128
KL = L_PAD // 128       # 16 link chunks
KG = 4                  # k-groups for the head pipeline (4 chunks each)
KPG = KL // KG
FB = L_PAD // 256       # 8 link blocks in the folded epilogue layout
EPS = 1e-12


def _build_program(PSHARD):
    """Emit the SPMD Bass program (identical on all cores)."""
    NCH = PSHARD // 128                  # even (host guarantees)
    NPS = (PSHARD + 511) // 512          # 512-path blocks
    NPR = NCH // 2                       # chunk pairs for fp8 DoubleRow mm2
    nc = bacc.Bacc("TRN2", target_bir_lowering=False, debug=False,
                   num_devices=NCORES)

    # ---- parameters (per-core shards) ----
    # features transposed: [link%128, (kg, f, kk, dh)] bf16
    p_ft = nc.dram_tensor("ftt", [128, KG * NF * KPG * DH], BF16,
                          kind="ExternalInput")
    # D block-major fp8: p_d[b][p, 512k + j] = D[128k+p, 512b+j]
    p_d = nc.dram_tensor("dkb", [NPS, 128, KL * 512], F8,
                         kind="ExternalInput")
    # D^T chunk-pair-major fp8: p_dt[t][p, c*L_PAD + l], c in {0,1}
    p_dt = nc.dram_tensor("dtp", [NPR, 128, 2 * L_PAD], F8,
                          kind="ExternalInput")
    # same-od 0/1 matrices, one per od-aligned chunk
    p_B = nc.dram_tensor("bod", [128, NCH * 128], F8, kind="ExternalInput")
    p_qs = nc.dram_tensor("qsp", [128, NCH], F32, kind="ExternalInput")
    # theta_links chunks (cols 0..KL-1) + theta_raw replicated (cols KL..)
    p_tc = nc.dram_tensor("tc", [128, KL + NF], F32, kind="ExternalInput")
    p_id = nc.dram_tensor("idn", [128, 128], F32, kind="ExternalInput")
    # folded (96, 256) epilogue tensors (host-replicated layouts, no math)
    p_kb = nc.dram_tensor("kb96", [DH, 256], F32, kind="ExternalInput")
    p_bb = nc.dram_tensor("bb96", [DH, 256], F32, kind="ExternalInput")
    p_lab = nc.dram_tensor("lab96", [DH, 256], F32, kind="ExternalInput")
    p_ttf = nc.dram_tensor("ttf96", [DH, 256], F32, kind="ExternalInput")
    p_out = nc.dram_tensor("out", [DH, 256], F32, kind="ExternalOutput")
    groups = [list(range(NCORES))]

    with tile.TileContext(nc) as tc:
        with tc.tile_pool(name="const", bufs=1) as cpool, \
             tc.tile_pool(name="dram", bufs=1, space="DRAM") as dpool, \
             tc.tile_pool(name="big", bufs=1) as bpool, \
             tc.tile_pool(name="stream", bufs=4) as spool, \
             tc.tile_pool(name="dkp", bufs=3) as dkpool, \
             tc.tile_pool(name="dtp", bufs=4) as dtpool:

            # warmup AllGather, fully dependency-free so it executes in
            # the quiet window BEFORE the DMA ring ramps (CC ops yield to
            # DMA traffic on this fabric -- with any late-landing input
            # the warmup floats to compute end and serializes in front of
            # the real AllToAll; without it the AllToAll pays ~20us of
            # first-collective cost)
            wgs = cpool.tile([1, 32], F32, tag="wgs")
            nc.vector.memset(wgs[:], 0.0)
            wg_in = dpool.tile([1, 32], F32, tag="wgi")
            nc.sync.dma_start(wg_in[:], wgs[:])
            wg_out = dpool.tile([1, 32 * NCORES], F32, tag="wgo")
            nc.gpsimd.collective_compute(
                "AllGather", ALU.bypass, replica_groups=groups,
                ins=[wg_in.opt()], outs=[wg_out.opt()])

            # ---- DMA ring.  mm1 block 0 is gated on dk0 + the g0 lhsT,
            # so the scalar queue carries exactly that critical prefix.
            # The sync ring is GATED on dk0's arrival (tiny dummy read)
            # and the gpsimd ring on dk1's, so the early wire belongs to
            # the critical transfers alone.
            tc_sb = cpool.tile([128, KL + NF], F32, tag="tc")
            nc.scalar.dma_start(tc_sb[:], p_tc.ap())
            GW = NF * KPG * DH               # cols per k-group in p_ft
            fts = bpool.tile([128, KG * GW], BF16, tag="fts")
            nc.scalar.dma_start(fts[:, 0:GW], p_ft.ap()[:, 0:GW])

            dk_t = [dkpool.tile([128, KL * 512], F8, tag="dk",
                                name=f"dk{b}") for b in range(NPS)]
            dtp_t = [dtpool.tile([128, 2 * L_PAD], F8, tag="dt",
                                 name=f"dt{t}") for t in range(NPR)]
            nc.scalar.dma_start(dk_t[0][:], p_d.ap()[0])
            ident = cpool.tile([128, 128], F32, tag="ident")
            nc.scalar.dma_start(ident[:], p_id.ap())
            nc.scalar.dma_start(dtp_t[0][:], p_dt.ap()[0])
            nc.scalar.dma_start(fts[:, GW:2 * GW], p_ft.ap()[:, GW:2 * GW])
            nc.scalar.dma_start(dtp_t[1][:], p_dt.ap()[1])

            # sync ring: gate on the g0 feature slab (small, lands early)
            # so dk0 owns the first wire window, then deadline order
            gate0 = dpool.tile([1, 32], BF16, tag="gate0")
            nc.sync.dma_start(gate0[:], fts[0:1, 0:32])
            if NPS > 1:
                nc.sync.dma_start(dk_t[1][:], p_d.ap()[1])
            b_all = cpool.tile([128, NCH * 128], F8, tag="b_all")
            nc.sync.dma_start(b_all[:, 0:NCH * 64], p_B.ap()[:, 0:NCH * 64])
            qs_sb = cpool.tile([128, NCH], F32, tag="qs")
            nc.sync.dma_start(qs_sb[:], p_qs.ap())
            for g in range(2, KG):
                nc.sync.dma_start(fts[:, g * GW:(g + 1) * GW],
                                  p_ft.ap()[:, g * GW:(g + 1) * GW])
            nc.sync.dma_start(b_all[:, NCH * 64:], p_B.ap()[:, NCH * 64:])
            kb = cpool.tile([DH, 256], F32, tag="kb")
            nc.sync.dma_start(kb[:], p_kb.ap())
            bb = cpool.tile([DH, 256], F32, tag="bb")
            nc.sync.dma_start(bb[:], p_bb.ap())
            lab = cpool.tile([DH, 256], F32, tag="lab")
            nc.sync.dma_start(lab[:], p_lab.ap())
            ttf = cpool.tile([DH, 256], F32, tag="ttf")
            nc.sync.dma_start(ttf[:], p_ttf.ap())
            # remaining dk blocks and dt pairs, ordered by use deadline,
            # issued from the (otherwise idle) gpsimd queue; gated on the
            # g1 feature slab so the early wire stays uncontended
            gate1 = dpool.tile([1, 32], BF16, tag="gate1")
            nc.gpsimd.dma_start(gate1[:], fts[0:1, GW:GW + 32])
            later = [("dk", b) for b in range(2, NPS)]
            for t in range(2, NPR):
                later.insert(min(2 * (t - 2) + 1, len(later)), ("dt", t))
            for kind, i in later:
                if kind == "dk":
                    nc.gpsimd.dma_start(dk_t[i][:], p_d.ap()[i])
                else:
                    nc.gpsimd.dma_start(dtp_t[i][:], p_dt.ap()[i])

            thb = cpool.tile([128, NF], F32, tag="thb")
            nc.vector.tensor_scalar_min(thb[:], tc_sb[:, KL:KL + NF], 0.0)

            with tc.tile_pool(name="psA", bufs=2, space="PSUM") as psA:
                # lhsT build, k-group pipelined with the feature DMA:
                # v0t[l, (k, dh)] = sum_f theta_f * featsT (bf16, 2x DVE
                # rate), then packed to lh[l, (k, dh..+tl)] with
                # theta_links in col 96 of each chunk.  Packs for the
                # early groups ride the vector queue (scalar is still
                # issuing DMAs), later groups on scalar.
                v0t = bpool.tile([128, KL * DH], BF16, tag="v0t")
                lh = bpool.tile([128, KL * 97], BF16, tag="lh")
                GV = KPG * DH                # v0t cols per k-group
                for g in range(KG):
                    base = g * GW
                    nc.vector.tensor_scalar_mul(
                        v0t[:, g * GV:(g + 1) * GV],
                        fts[:, base:base + GV], thb[:, 0:1])
                    for f in range(1, NF):
                        nc.vector.scalar_tensor_tensor(
                            v0t[:, g * GV:(g + 1) * GV],
                            fts[:, base + f * GV:base + (f + 1) * GV],
                            thb[:, f:f + 1],
                            v0t[:, g * GV:(g + 1) * GV], ALU.mult, ALU.add)
                    lh3 = lh[:].rearrange("p (k j) -> p k j", j=97)
                    pk = nc.vector.tensor_copy if g < 2 else nc.scalar.copy
                    pk(lh3[:, g * KPG:(g + 1) * KPG, 0:DH],
                       v0t[:, g * GV:(g + 1) * GV].rearrange(
                           "p (k j) -> p k j", j=DH))
                    pk(lh3[:, g * KPG:(g + 1) * KPG, DH:DH + 1],
                       tc_sb[:, g * KPG:(g + 1) * KPG].rearrange(
                           "p (k j) -> p k j", j=1))

                # ---- the block pipeline ----
                ysb = bpool.tile([97, PSHARD], F32, tag="ysb")
                evb = bpool.tile([128, DH * NCH], BF16, tag="evb")
                ft8 = bpool.tile([128, DH * NCH], F8, tag="ft8")
                qsq = cpool.tile([128, NCH], F32, tag="qsq")
                nc.vector.tensor_mul(qsq[:], qs_sb[:], qs_sb[:])
                ar_in = dpool.tile([DH, L_PAD], BF16, tag="arin")
                ar_aa = dpool.tile([DH, L_PAD], BF16, tag="araa")
                # epilogue constants (inputs land ~20us; emitted mid-ring
                # below so the queues never stall on them)
                ib = cpool.tile([DH, 256], F32, tag="ib")
                bb2 = cpool.tile([DH, 256], F32, tag="bb2")
                ab = cpool.tile([DH, 256], F32, tag="ab")
                atf = cpool.tile([DH, 256], F32, tag="atf")

                def chunk_softmax(c):
                    """transpose -> exp for path chunk c."""
                    yt_ps = psA.tile([128, 97], F32, tag="m",
                                     name=f"yt{c}")
                    nc.tensor.matmul(yt_ps[:], ysb[:, 128 * c:128 * (c + 1)],
                                     ident[:97, :97], is_transpose=True,
                                     start=True, stop=True)
                    cvec = spool.tile([128, 1], F32, tag="cvec")
                    nc.vector.tensor_copy(cvec[:], yt_ps[:, DH:DH + 1])
                    nc.scalar.activation(evb[:, DH * c:DH * (c + 1)],
                                         yt_ps[:, 0:DH], AF.Exp, bias=cvec[:])

                def chunk_flow(c):
                    """in-chunk denominator matmul + path flows for chunk c;
                    fp8 DoubleRow matmul2 on each odd c for the pair
                    (c-1, c)."""
                    g_ps = psA.tile([128, DH], F32, tag="m", name=f"g{c}")
                    nc.tensor.matmul(g_ps[:],
                                     b_all[:, 128 * c:128 * (c + 1)],
                                     evb[:, DH * c:DH * (c + 1)],
                                     start=True, stop=True)
                    rec = spool.tile([128, DH], F32, tag="rec")
                    nc.vector.reciprocal_approx_fast(out=rec[:], in_=g_ps[:])
                    nc.vector.scalar_tensor_tensor(
                        ft8[:, DH * c:DH * (c + 1)],
                        evb[:, DH * c:DH * (c + 1)],
                        qsq[:, c:c + 1], rec[:], ALU.mult, ALU.mult)
                    if c % 2 == 1:
                        t = c // 2
                        lhs3 = ft8[:].rearrange(
                            "p (c m) -> p c m", m=DH)[:, c - 1:c + 1, :]
                        rhs3 = dtp_t[t][:].rearrange(
                            "p (c l) -> p c l", c=2)
                        for n in range(L_PAD // 512):
                            nc.tensor.matmul(
                                x_ps[n][:], lhs3,
                                rhs3[:, :, 512 * n:512 * (n + 1)],
                                perf_mode=DR,
                                start=(c == 1), stop=(c == NCH - 1))

                with tc.tile_pool(name="psV", bufs=2, space="PSUM") as psV, \
                     tc.tile_pool(name="psX", bufs=1, space="PSUM") as psX:
                    x_ps = [psX.tile([DH, 512], F32, tag=f"x{n}",
                                     name=f"x{n}")
                            for n in range(L_PAD // 512)]
                    for b in range(NPS):
                        w = min(512, PSHARD - 512 * b)
                        vf_ps = psV.tile([97, w], F32, tag="vf",
                                         name=f"vf{b}")
                        for k in range(KL):
                            nc.tensor.matmul(
                                vf_ps[:], lh[:, 97 * k:97 * (k + 1)],
                                dk_t[b][:, 512 * k:512 * k + w],
                                start=(k == 0), stop=(k == KL - 1))
                        if b % 2 == 0:
                            nc.scalar.copy(ysb[:, 512 * b:512 * b + w],
                                           vf_ps[:])
                        else:
                            nc.vector.tensor_copy(
                                ysb[:, 512 * b:512 * b + w], vf_ps[:])
                        for c in range(4 * b, 4 * b + w // 128):
                            chunk_softmax(c)
                            chunk_flow(c)
                        if b == min(3, NPS - 1):
                            # epilogue constant prep, hidden under compute
                            # (late enough that the const DMAs have landed)
                            nc.vector.reciprocal_approx_fast(
                                out=ib[:], in_=kb[:])
                            nc.vector.tensor_scalar(bb2[:], bb[:],
                                                    float(EPS), 4.0,
                                                    ALU.max, ALU.min)
                            nc.scalar.activation(ab[:], lab[:], AF.Exp)
                            nc.vector.tensor_mul(atf[:], ab[:], ttf[:])

                    # force the Ln table load NOW (hidden under the
                    # drain + AllToAll) instead of on the tail chain
                    scr = spool.tile([128, 1], F32, tag="scr")
                    nc.scalar.activation(scr[:], qs_sb[:, 0:1], AF.Ln)

                    # drain: PSUM -> bf16 -> DRAM -> one AllToAll
                    xb = bpool.tile([DH, L_PAD], BF16, tag="xb")
                    for n in range(L_PAD // 512):
                        if n % 2 == 0:
                            nc.scalar.copy(xb[:, 512 * n:512 * (n + 1)],
                                           x_ps[n][:])
                        else:
                            nc.vector.tensor_copy(
                                xb[:, 512 * n:512 * (n + 1)], x_ps[n][:])
                        nc.sync.dma_start(
                            ar_in[:, 512 * n:512 * (n + 1)],
                            xb[:, 512 * n:512 * (n + 1)])
                    nc.gpsimd.collective_compute(
                        "AllToAll", ALU.bypass,
                        replica_groups=groups,
                        ins=[ar_in.opt()], outs=[ar_aa.opt()])

                # ---- gather the 8 incoming partials in the folded
                # (96, 256) layout (one fold DMA per peer, issue cost
                # split across the sync + scalar queues), then an add
                # tree split across vector + gpsimd ----
                # NOTE: this must stay one DMA per peer with a 3D source
                # AP and a plain 2D dest -- every fancier multi-peer AP
                # form (split dest partition dim, j-outer 3D dest) lowers
                # to silently wrong descriptors on this stack.
                # fold DMAs ride sync+scalar only, so the gpsimd queue is
                # free to start its half of the add tree the moment its
                # slices land
                xga = bpool.tile([DH, NCORES * 256], BF16, tag="xga")
                for j in range(NCORES):
                    eng = (nc.sync, nc.scalar)[j % 2]
                    eng.dma_start(
                        xga[:, 256 * j:256 * (j + 1)],
                        ar_aa[DHS * j:DHS * (j + 1), :].rearrange(
                            "d (a l) -> (d a) l", a=FB))
                xs0 = bpool.tile([DH, 256], F32, tag="xs0")
                nc.vector.tensor_add(xs0[:], xga[:, 0:256], xga[:, 256:512])
                for j in (2, 3):
                    nc.vector.tensor_add(
                        xs0[:], xs0[:], xga[:, 256 * j:256 * (j + 1)])
                xs1 = bpool.tile([DH, 256], F32, tag="xs1")
                nc.gpsimd.tensor_add(xs1[:], xga[:, 1024:1280],
                                     xga[:, 1280:1536])
                for j in (6, 7):
                    nc.gpsimd.tensor_add(
                        xs1[:], xs1[:], xga[:, 256 * j:256 * (j + 1)])

                # ---- BPR epilogue in the folded (96, 256) layout ----
                # (max before add is equivalent here: both sides >= 0)
                t0 = bpool.tile([DH, 256], F32, tag="t0")
                nc.vector.scalar_tensor_tensor(
                    t0[:], xs0[:], 1e-35, xs1[:], ALU.max, ALU.add)
                nc.vector.tensor_mul(t0[:], t0[:], ib[:])
                t1 = bpool.tile([DH, 256], F32, tag="t1")
                nc.scalar.activation(t1[:], t0[:], AF.Ln)
                nc.vector.tensor_mul(t1[:], t1[:], bb2[:])
                t2 = bpool.tile([DH, 256], F32, tag="t2")
                nc.scalar.activation(t2[:], t1[:], AF.Exp)
                nc.vector.tensor_mul(t2[:], t2[:], atf[:])
                o_t = bpool.tile([DH, 256], F32, tag="o")
                nc.vector.tensor_add(o_t[:], t2[:], ttf[:])
                nc.sync.dma_start(p_out.ap(), o_t[:])

    nc.compile()
    return nc


_CACHE = {}
LAST_RESULT = None


def _get_program(PSHARD):
    if PSHARD not in _CACHE:
        _CACHE[PSHARD] = _build_program(PSHARD)
    return _CACHE[PSHARD]


def _pack_ods(odl):
    """Best-fit-decreasing packing of whole ods into 128-path chunks.
    Returns (slots, n_chunks): slots[j] is the padded position of local
    path j (odl is sorted, paths of one od contiguous)."""
    uods, ucnts = np.unique(odl, return_counts=True)
    order = np.argsort(-ucnts, kind="stable")
    cap = []                      # remaining capacity per chunk
    place = np.empty((len(uods), 2), np.int64)
    for oi in order:
        cnt = int(ucnts[oi])
        best, bestrem = -1, 129
        for bi, r in enumerate(cap):
            if cnt <= r < bestrem:
                best, bestrem = bi, r
        if best < 0:
            cap.append(128)
            best = len(cap) - 1
        place[oi] = (best, 128 - cap[best])
        cap[best] -= cnt
    starts = np.concatenate([[0], np.cumsum(ucnts)[:-1]])
    slots = np.empty(len(odl), np.int64)
    for oi in range(len(uods)):
        bi, off = place[oi]
        cnt = ucnts[oi]
        slots[starts[oi]:starts[oi] + cnt] = bi * 128 + off + np.arange(cnt)
    return slots, len(cap)


def _fold96(v_lpad):
    """(L_PAD,) per-link vector -> (96, 256) folded layout (row 8*d + a holds
    link block [256a, 256(a+1)) for every local day-hour d)."""
    return np.ascontiguousarray(
        np.tile(v_lpad.reshape(FB, 256), (DHS, 1)).astype(np.float32))


def kernel(X, theta_raw, theta_links, q_sqrt, log_alpha, beta_raw, k, D,
           od_of_path, n_ods):
    X = np.asarray(X, np.float32)
    D = np.asarray(D, np.float32)
    od = np.asarray(od_of_path, np.int32)
    assert X.shape == (ND, NH, NL, NF + 1) and D.shape == (NL, NP)
    assert int(n_ods) == NOD

    # core bounds: equal PATH counts, snapped to od boundaries (keeps the
    # per-core chunk count at its minimum)
    bounds = np.empty(NCORES + 1, np.int64)
    bounds[0], bounds[-1] = 0, NP
    for i in range(1, NCORES):
        idx = i * NP // NCORES
        bounds[i] = np.searchsorted(od, od[min(idx, NP - 1)])

    # ---- od-aligned chunk packing (index bookkeeping only) ----
    slot_maps = []
    nch_need = 0
    for i in range(NCORES):
        slots, nch = _pack_ods(od[bounds[i]:bounds[i + 1]])
        slot_maps.append(slots)
        nch_need = max(nch_need, nch)
    NCH = int(np.ceil(nch_need / 2) * 2)     # even for mm2 pairs
    PSHARD = NCH * 128
    NPS = (PSHARD + 511) // 512

    nc = _get_program(PSHARD)

    F8H = ml_dtypes.float8_e4m3fn

    # ---- host-side shard construction (index bookkeeping + relayout only) --
    Xf = X.reshape(DH, NL, NF + 1)
    ttf_full = np.zeros((DH, L_PAD), np.float32)
    ttf_full[:, :NL] = Xf[:, :, 0]
    # featsT[link%128, (kg, f, kk, dh)] bf16
    ftt = np.zeros((L_PAD, NF, DH), np.float32)
    for f in range(NF):
        ftt[:NL, f, :] = Xf[:, :, f + 1].T
    ftt = (ftt.reshape(KG, KPG, 128, NF, DH).transpose(2, 0, 3, 1, 4)
           .reshape(128, KG * NF * KPG * DH))
    ftt_h = np.ascontiguousarray(ftt).astype(ml_dtypes.bfloat16)

    def padded_vec(v, fill=0.0):
        o = np.full(L_PAD, fill, np.float32)
        o[:NL] = v
        return o

    tc_h = np.concatenate(
        [padded_vec(np.asarray(theta_links, np.float32)).reshape(KL, 128).T,
         np.tile(np.asarray(theta_raw, np.float32), (128, 1))], axis=1)
    tc_h = np.ascontiguousarray(tc_h)
    kb_h = _fold96(padded_vec(np.asarray(k, np.float32), fill=1.0))
    bb_h = _fold96(padded_vec(np.asarray(beta_raw, np.float32)))
    lab_h = _fold96(padded_vec(np.asarray(log_alpha, np.float32)))
    qsr = np.asarray(q_sqrt, np.float32)
    id_h = np.eye(128, dtype=np.float32)

    in_maps = []
    for i in range(NCORES):
        lo, hi = bounds[i], bounds[i + 1]
        odl = od[lo:hi]
        slots = slot_maps[i]

        PB = NPS * 512
        Dsh = np.zeros((L_PAD, PB), np.float32)
        Dsh[:NL, slots] = D[:, lo:hi]
        # block-major D fp8: dkb[b][p, 512k + j] = D[128k+p, 512b+j]
        dkb = np.ascontiguousarray(
            Dsh.reshape(KL, 128, NPS, 512).transpose(2, 1, 0, 3)
            .reshape(NPS, 128, KL * 512)).astype(F8H)
        # chunk-pair-major D^T fp8: dtp[t][p, c*L_PAD + l] = D^T[(2t+c)*128+p, l]
        dtp = np.ascontiguousarray(
            Dsh.T[:PSHARD].astype(F8H).reshape(NCH // 2, 2, 128, L_PAD)
            .transpose(0, 2, 1, 3).reshape(NCH // 2, 128, 2 * L_PAD))

        # same-od 0/1 matrices (pure index bookkeeping); od-aligned chunks
        # mean no od crosses a chunk boundary
        odp = np.full(NCH * 128, -1, np.int64)
        odp[slots] = odl
        oc = odp.reshape(NCH, 128)
        b_h = np.zeros((128, NCH, 128), F8H)
        for c in range(NCH):
            b_h[:, c, :] = (oc[c][:, None] == oc[c][None, :])

        qs_h = np.zeros(NCH * 128, np.float32)
        qs_h[slots] = qsr[odl]
        qs_h = np.ascontiguousarray(qs_h.reshape(NCH, 128).T)

        in_maps.append(dict(
            ftt=ftt_h, dkb=dkb, dtp=dtp,
            bod=np.ascontiguousarray(b_h.reshape(128, NCH * 128)),
            qsp=qs_h, tc=tc_h, idn=id_h,
            kb96=kb_h, bb96=bb_h, lab96=lab_h,
            ttf96=np.ascontiguousarray(
                ttf_full[DHS * i:DHS * (i + 1)].reshape(DH, 256))))

    trace = os.environ.get("BASS_KERNEL_TRACE", "0") == "1"
    global LAST_RESULT
    for _attempt in range(3):
        res = run_bass_kernel_spmd(nc, in_maps, core_ids=list(range(NCORES)),
                                   trace=trace)
        LAST_RESULT = res
        parts = [r["out"].reshape(DHS, L_PAD) for r in res.results]
        out = np.concatenate(parts, axis=0)[:, :NL]
        if np.isfinite(out).all():
            break
    return np.ascontiguousarray(out).reshape(ND, NH, NL).astype(np.float32)
